# revision 1
# baseline (speedup 1.0000x reference)
"""HGT layer on 8 Trainium2 NeuronCores.

Sharding: 1D-partition DESTINATION nodes across the 8 cores (spec hint). Each
core owns a contiguous dst range, so per-(dst,etype) segment softmax is fully
core-local. Node features/params are replicated; each core computes K/V for
all nodes it may gather from.

The ragged per-(dst,etype) segments are turned into DENSE tensors on the
device: host buckets each core's edges into a [n_per, deg] slot grid (padded)
plus a one-hot [n_per, deg, R] etype mask — pure index prep. The device then
does masked max / sum einsums instead of segment_* scatter ops (which the
Neuron XLA bridge cannot lower). The only indexed op on device is the k/v row
gather by src, which lowers to DGE descriptors.
"""
import numpy as np
import jax
import jax.numpy as jnp

N_CORES = 8
N = 50000
H, DK, R, T = 8, 32, 8, 4
OUT_DIM = H * DK
IN_DIM = 256


def _shard_edges_dense(src, dst, etype):
    """Bucket edges by dst-range shard, then lay each core's edges out as a
    dense [n_per, deg] slot grid. Returns per-core src grid, one-hot etype
    mask [n_per, deg, R], and n_per."""
    n_per = N // N_CORES
    e = len(src)
    # degree per dst node (global)
    deg_all = np.bincount(dst, minlength=N)
    deg = int(deg_all.max())
    # slot index of each edge within its dst node
    order = np.argsort(dst, kind="stable")
    starts = np.zeros(N, np.int64)
    starts[1:] = np.cumsum(deg_all)[:-1]
    slot = np.empty(e, np.int64)
    slot[order] = np.arange(e) - starts[dst[order]]

    src_grid = np.zeros((N, deg), np.int32)
    et_grid = np.full((N, deg), -1, np.int32)   # -1 = padding slot
    src_grid[dst, slot] = src
    et_grid[dst, slot] = etype

    src_grid = src_grid.reshape(N_CORES, n_per, deg)
    et_grid = et_grid.reshape(N_CORES, n_per, deg)
    # one-hot over R, zeros for padding
    oh = (et_grid[..., None] == np.arange(R)).astype(np.float32)
    return src_grid, oh, n_per, deg


def _core_fn(src_g, oh, x_own, nt_own, x, node_type,
             Wk, bk, Wq, bq, Wv, bv, Wa, ba,
             rel_att, rel_msg, rel_pri, skip):
    n_per, deg = src_g.shape
    sqrt_dk = jnp.asarray(np.sqrt(DK), jnp.float32)

    def typed_linear(xx, nt, W, b):
        out = jnp.zeros((xx.shape[0], W.shape[2]), dtype=xx.dtype)
        for t in range(T):
            y = xx @ W[t] + b[t]
            out = jnp.where((nt == t)[:, None], y, out)
        return out

    k = typed_linear(x, node_type, Wk, bk)              # [N, 256]
    v = typed_linear(x, node_type, Wv, bv)
    q = typed_linear(x_own, nt_own, Wq, bq).reshape(n_per, H, DK)

    sf = src_g.reshape(-1)                               # [n_per*deg]
    k_e = k[sf].reshape(n_per, deg, H, DK)
    v_e = v[sf].reshape(n_per, deg, H, DK)

    # dst-side relation transform: <rel_att[r,h] k, q> = <k, rel_att[r,h]^T q>
    q_r = jnp.einsum('nhd,rhde->nrhe', q, rel_att)       # [n_per, R, H, DK]
    q_e = jnp.einsum('nsr,nrhe->nshe', oh, q_r)          # per-slot selected q_r
    pri = jnp.einsum('nsr,rh->nsh', oh, rel_pri)

    att = (q_e * k_e).sum(-1) * pri / sqrt_dk            # [n_per, deg, H]

    # masked softmax per (n, r) over slots
    ohm = oh[..., None]                                   # [n, s, R, 1]
    neg = jnp.asarray(-1e30, jnp.float32)
    att4 = att[:, :, None, :]                             # [n, s, 1, h]
    m = jnp.where(ohm > 0, att4, neg).max(axis=1)         # [n, R, h]
    m_sel = jnp.einsum('nsr,nrh->nsh', oh, m)             # 0 for pad slots
    ex = jnp.exp(att - m_sel)                             # [n, s, h]
    den = jnp.einsum('nsr,nsh->nrh', oh, ex)              # [n, R, h]
    den_sel = jnp.einsum('nsr,nrh->nsh', oh, den)
    alpha = ex / jnp.where(den_sel > 0, den_sel, 1.0)     # pad slots -> ex, masked next

    wmsg = jnp.einsum('nsr,nshe->nrhe', oh, alpha[..., None] * v_e)
    hmsg = jnp.einsum('nrhd,rhde->nrhe', wmsg, rel_msg).reshape(n_per, R, OUT_DIM)

    present = oh.max(axis=1)                              # [n, R]
    cnt = jnp.maximum(present.sum(axis=1, keepdims=True), 1.0)
    t_agg = hmsg.sum(axis=1) / cnt                        # [n, 256]

    trans = typed_linear(t_agg, nt_own, Wa, ba)
    a = jax.nn.sigmoid(skip)[nt_own][:, None]
    return trans * a + x_own * (1.0 - a)


_pmapped = jax.pmap(_core_fn, in_axes=(0, 0, 0, 0) + (None,) * 14)


def kernel(**inputs):
    x = np.asarray(inputs["x"], np.float32)
    node_type = np.asarray(inputs["node_type"], np.int32)
    src = np.asarray(inputs["src"], np.int32)
    dst = np.asarray(inputs["dst"], np.int32)
    etype = np.asarray(inputs["etype"], np.int32)

    src_g, oh, n_per, deg = _shard_edges_dense(src, dst, etype)
    x_own = x.reshape(N_CORES, n_per, IN_DIM)
    nt_own = node_type.reshape(N_CORES, n_per)

    out = _pmapped(
        jnp.asarray(src_g), jnp.asarray(oh),
        jnp.asarray(x_own), jnp.asarray(nt_own),
        jnp.asarray(x), jnp.asarray(node_type),
        jnp.asarray(inputs["Wk"]), jnp.asarray(inputs["bk"]),
        jnp.asarray(inputs["Wq"]), jnp.asarray(inputs["bq"]),
        jnp.asarray(inputs["Wv"]), jnp.asarray(inputs["bv"]),
        jnp.asarray(inputs["Wa"]), jnp.asarray(inputs["ba"]),
        # dst-side transform needs rel_att^T per (r,h): <A k, q> = <k, A^T q>.
        # Transposing on host keeps the compiled HLO identical (cache hit).
        jnp.asarray(np.ascontiguousarray(
            np.asarray(inputs["rel_att"], np.float32).transpose(0, 1, 3, 2))),
        jnp.asarray(inputs["rel_msg"]),
        jnp.asarray(inputs["rel_pri"]), jnp.asarray(inputs["skip"]),
    )
    return np.asarray(out).reshape(N, OUT_DIM).astype(np.float32)

